# revision 19
# baseline (speedup 1.0000x reference)
"""DeepGMR (segment_reduce) Trainium2 kernel.

Problem: B=8, D=3, N=65536, K=64.
    logits = W^T @ pts  (per batch, both template & source)
    gamma  = softmax over N
    mu_k   = sum_n gamma * pts / Npi ; 3x3 Procrustes; aligned = src @ R^T + T
sigma/Exx in the reference are dead code (never used by the output).

Sharding: batch-parallel, core c <-> batch b=c. No collectives.

Device kernel A (per core): sufficient statistics
    P[k, c] = sum_n exp(l[k, n]) * pts4[c, n],  pts4 = (x, y, z, 1)
  computed as:
    - logits via fp32r matmuls, stationary = W halves [3, 32] placed at
      row-band of the point quarter, 4 col-band-packed tiles per 512-pt
      step -> PSUM [128, 512] = (k-half x quarter-parity) x n
    - ACT exp over 3-bank chunks PSUM->SBUF bf16 (big free dim amortizes
      the +352cyc ACT overhead)
    - DVE 32x32 block transpose (bf16)
    - per 32-block matmuls: lhsT = host-prepared pts4 tiles [32, 16]
      (4-merged: 4 blocks -> out [16, 128], useful diagonal extracted on
      host), accumulated in one PSUM bank.
Host: mu = (P/S)/(1+2eps), exact reference Procrustes (SVD + det flip).
Device kernel B (per core): aligned = R @ src + T on DVE/ACT in
[128, 512] partition-major layout; host reassembles (B, N, 3).
"""

import numpy as np
import ml_dtypes

import concourse.bass as bass
import concourse.tile as tile
from concourse import bacc, mybir
from concourse.bass_utils import run_bass_kernel_spmd

B, D, N, K = 8, 3, 65536, 64
NCORES = 8
EPS = 1e-8
Q = N // 4            # 16384 points per quarter (per partition-group)
NSTEP = Q // 512      # 32 matmul steps per pass
CHUNK_STEPS = [2] * 16   # ACT/transpose chunk sizes (sum = 32)
assert sum(CHUNK_STEPS) == NSTEP

F32 = mybir.dt.float32
F32R = mybir.dt.float32r
BF16 = mybir.dt.bfloat16

_CACHE = {}


# --------------------------------------------------------------------------
# Kernel A: sufficient statistics
# --------------------------------------------------------------------------
def _build_kernel_a():
    nc = bacc.Bacc("TRN2", target_bir_lowering=False, debug=False,
                   num_devices=NCORES)
    tf_d = nc.dram_tensor("tf", [D, N], BF16, kind="ExternalInput").ap()
    sf_d = nc.dram_tensor("sf", [D, N], BF16, kind="ExternalInput").ap()
    w_d = nc.dram_tensor("w_rep", [8, 128], BF16, kind="ExternalInput").ap()
    p4_d = nc.dram_tensor("pts4m", [128, 16384], BF16, kind="ExternalInput").ap()
    stats_d = nc.dram_tensor("stats", [32, 512], F32, kind="ExternalOutput").ap()

    with tile.TileContext(nc) as tc:
        with (
            tc.tile_pool(name="const", bufs=1) as cpool,
            tc.tile_pool(name="stage", bufs=3) as spool,
            tc.tile_pool(name="esb", bufs=4) as epool,
            tc.tile_pool(name="btp", bufs=4) as bpool,
            tc.tile_pool(name="stat", bufs=1) as stpool,
            tc.tile_pool(name="lps", bufs=2, space=bass.MemorySpace.PSUM) as lpsum,
            tc.tile_pool(name="pacc", bufs=2, space=bass.MemorySpace.PSUM) as ppsum,
        ):
            w_sb = cpool.tile([8, 128], BF16)
            nc.sync.dma_start(out=w_sb[:, :], in_=w_d[:, :])
            p4z = cpool.tile([128, 128], BF16)
            nc.sync.dma_start(out=p4z[:, :], in_=p4_d[:, 0:128])
            zbias = cpool.tile([128, 1], F32)
            nc.vector.memset(zbias[:, :], 0.0)
            z64 = cpool.tile([128, 32], BF16)
            nc.vector.memset(z64[:, :], 0.0)
            stats_sb = stpool.tile([32, 512], F32)

            for t, feat_d in enumerate((tf_d, sf_d)):
                pacc0 = ppsum.tile([32, 128], F32, tag="pacc0")
                pacc1 = ppsum.tile([32, 128], F32, tag="pacc1")
                nc.tensor.matmul(pacc0[:, :], z64[0:64, :], p4z[0:64, 0:128],
                                 start=True, stop=False, skip_group_check=True)
                nc.tensor.matmul(pacc1[:, :], z64[64:128, :], p4z[64:128, 0:128],
                                 start=True, stop=False, skip_group_check=True,
                                 tile_position=(64, 0))
                for sc in range(2):        # superchunks of 16384 per half
                    # halves of N at partition rows 0-2 (H0) and 4-6 (H1)
                    stage = spool.tile([128, Q], BF16, tag="stage")
                    nc.sync.dma_start(
                        out=stage[0:3, :],
                        in_=feat_d[:, Q * sc : Q * (sc + 1)])
                    nc.sync.dma_start(
                        out=stage[4:7, :],
                        in_=feat_d[:, 2 * Q + Q * sc : 2 * Q + Q * (sc + 1)])
                    # row 3 is a gap in the [7, N] contract; W row 3 is zero
                    # but the data must be finite (0*Inf = NaN), so fill it.
                    nc.sync.dma_start(
                        out=stage[3:4, :], in_=feat_d[0:1, 0:Q])
                    p4p = spool.tile([128, 4096], BF16, tag="p4p")
                    nc.sync.dma_start(
                        out=p4p[:, :],
                        in_=p4_d[:, 4096 * (2 * t + sc) : 4096 * (2 * t + sc + 1)])
                    s0 = 0
                    for csteps in CHUNK_STEPS:
                        nfree = 512 * csteps
                        chunk = lpsum.tile([128, 1024], F32, tag="lchunk")
                        # logits: ONE [7,128]x[7,512] matmul per 512-pt step
                        # (block-diagonal W: rows 0-2 -> k cols 0-63 for H0,
                        #  rows 4-6 -> cols 64-127 for H1)
                        for sl in range(csteps):
                            s = s0 + sl
                            fs = slice(512 * s, 512 * (s + 1))
                            ps = slice(512 * sl, 512 * (sl + 1))
                            nc.tensor.matmul(
                                chunk[:, ps],
                                w_sb[0:7, :], stage[0:7, fs],
                                start=True, stop=True)
                        e_sb = epool.tile([128, 1024], BF16, tag="esb")
                        nc.scalar.activation(
                            e_sb[:, :nfree], chunk[:, :nfree],
                            mybir.ActivationFunctionType.Exp,
                            bias=zbias[:, :])
                        bt = bpool.tile([128, 1024], BF16, tag="btp")
                        nc.vector.transpose(bt[:, :nfree], e_sb[:, :nfree])
                        # P: contract 64 (both k-halves), block-diag pts4
                        # tiles [64, 32], rhs [64, 128], out [32, 128]
                        for b4 in range(4 * csteps):
                            t4 = 4 * s0 + b4
                            col = 32 * t4
                            for h, pac in ((0, pacc0), (1, pacc1)):
                                nc.tensor.matmul(
                                    pac[0:32, 0:128],
                                    p4p[64 * h : 64 * h + 64, col : col + 32],
                                    bt[64 * h : 64 * h + 64,
                                       128 * b4 : 128 * (b4 + 1)],
                                    start=False, stop=False,
                                    skip_group_check=True,
                                    tile_position=(64 * h, 0))
                        s0 += csteps
                nc.vector.tensor_copy(stats_sb[:, 256 * t : 256 * t + 128],
                                      pacc0[:, :])
                nc.vector.tensor_copy(stats_sb[:, 256 * t + 128 : 256 * t + 256],
                                      pacc1[:, :])
            nc.sync.dma_start(out=stats_d[:, :], in_=stats_sb[:, :])
    nc.compile()
    return nc


# --------------------------------------------------------------------------
# Kernel B: aligned = R @ src + T
# --------------------------------------------------------------------------
def _build_kernel_b():
    nc = bacc.Bacc("TRN2", target_bir_lowering=False, debug=False,
                   num_devices=NCORES)
    src_d = nc.dram_tensor("src3", [D, 128, 512], F32, kind="ExternalInput").ap()
    r_d = nc.dram_tensor("rmat", [128, 12], F32, kind="ExternalInput").ap()
    out_d = nc.dram_tensor("aligned3", [D, 128, 512], F32,
                           kind="ExternalOutput").ap()

    with tile.TileContext(nc) as tc:
        with tc.tile_pool(name="sb", bufs=1) as pool:
            r_sb = pool.tile([128, 12], F32)
            nc.sync.dma_start(out=r_sb[:, :], in_=r_d[:, :])
            comp = []
            for c in range(D):
                ct = pool.tile([128, 512], F32, tag=f"in{c}")
                nc.sync.dma_start(out=ct[:, :], in_=src_d[c, :, :])
                comp.append(ct)
            for d in range(D):
                acc = pool.tile([128, 512], F32, tag=f"acc{d}")
                # acc = (x * R[d,0]) + T[d]  (one DVE tensor_scalar, 2 scalars)
                nc.vector.tensor_scalar(
                    acc[:, :], comp[0][:, :],
                    r_sb[:, 4 * d : 4 * d + 1],
                    r_sb[:, 4 * d + 3 : 4 * d + 4],
                    op0=mybir.AluOpType.mult,
                    op1=mybir.AluOpType.add)
                for e in (1, 2):
                    # acc = (comp[e] * R[d,e]) + acc
                    nc.vector.scalar_tensor_tensor(
                        acc[:, :], comp[e][:, :],
                        r_sb[:, 4 * d + e : 4 * d + e + 1],
                        acc[:, :],
                        op0=mybir.AluOpType.mult,
                        op1=mybir.AluOpType.add)
                nc.sync.dma_start(out=out_d[d, :, :], in_=acc[:, :])
    nc.compile()
    return nc


# --------------------------------------------------------------------------
# Host-side helpers
# --------------------------------------------------------------------------
def _make_pts4m(feats):
    """feats (D, N) fp32 -> [128, 8192] bf16: block-diag [64, 32] pts4
    tiles (rows 0-31 -> cols 0-15 for kh0, rows 32-63 -> cols 16-31)."""
    pts4 = np.concatenate([feats, np.ones((1, N), np.float32)], axis=0)  # (4,N)
    out = np.zeros((128, 8192), dtype=np.float32)
    for h in range(2):
        for sc in range(2):
            q = 2 * h + sc
            qd = pts4[:, Q * q : Q * (q + 1)]          # (4, 16384)
            X = qd.reshape(4, 128, 4, 32)              # [c, T4, m, j]
            vals = X.transpose(3, 1, 2, 0).reshape(32, 128, 16)  # [j, T4, mc]
            blk = np.zeros((64, 128, 32), np.float32)
            blk[0:32, :, 0:16] = vals
            blk[32:64, :, 16:32] = vals
            out[64 * h : 64 * h + 64, 4096 * sc : 4096 * (sc + 1)] = (
                blk.reshape(64, 4096))
    return out.astype(ml_dtypes.bfloat16)


def _extract_PS(stats_half):
    """stats_half [32, 256] fp32 (bank H0 | bank H1) -> P (K, 3), S (K,)."""
    P4 = np.zeros((K, 4), np.float64)
    st = stats_half.astype(np.float64)
    for h in range(2):
        reg = st[:, 128 * h : 128 * h + 128]           # [32, 128]
        for khp in range(2):
            for m in range(4):
                blk = reg[16 * khp + 4 * m : 16 * khp + 4 * (m + 1),
                          32 * m : 32 * (m + 1)]       # [4c, 32i]
                P4[32 * khp : 32 * (khp + 1), :] += blk.T
    return P4[:, :3], P4[:, 3]


def _procrustes(Pt, St, Ps, Ss):
    """Exact reference tail. Returns R (3,3), T (3,)."""
    def mu_pi(P, S):
        mu = (P / S[:, None]) / (1.0 + 2 * EPS)
        pi = np.full(K, (1.0 + EPS) / N)
        return mu, pi
    t_mu, t_pi = mu_pi(Pt, St)
    s_mu, s_pi = mu_pi(Ps, Ss)
    cx = (s_mu * s_pi[:, None]).sum(0)
    cy = (t_mu * t_pi[:, None]).sum(0)
    mx = s_mu - cx
    my = t_mu - cy
    Wm = np.einsum('kd,ke->de', s_pi[:, None] * my, mx)
    U, _, Vh = np.linalg.svd(Wm)
    R0 = U @ Vh
    det = np.linalg.det(R0)
    Vh2 = Vh.copy()
    Vh2[:, -1] *= np.sign(det)
    R = U @ Vh2
    T = cy - R @ cx
    return R, T


def kernel(template_features, source_features, W):
    template_features = np.ascontiguousarray(
        np.asarray(template_features, dtype=np.float32))
    source_features = np.ascontiguousarray(
        np.asarray(source_features, dtype=np.float32))
    W = np.asarray(W, dtype=np.float32)

    if "a" not in _CACHE:
        _CACHE["a"] = _build_kernel_a()
        _CACHE["b"] = _build_kernel_b()
    nca, ncb = _CACHE["a"], _CACHE["b"]

    w_rep = np.zeros((8, 128), np.float32)
    w_rep[0:3, 0:64] = W
    w_rep[4:7, 64:128] = W
    w_rep = w_rep.astype(ml_dtypes.bfloat16)
    in_maps_a = []
    for b in range(B):
        in_maps_a.append({
            "tf": template_features[b].astype(ml_dtypes.bfloat16),
            "sf": source_features[b].astype(ml_dtypes.bfloat16),
            "w_rep": w_rep,
            "pts4m": np.concatenate(
                [_make_pts4m(template_features[b]),
                 _make_pts4m(source_features[b])], axis=1),
        })
    bkr_a = run_bass_kernel_spmd(nca, in_maps_a, list(range(NCORES)))
    res_a = bkr_a.results

    in_maps_b = []
    for b in range(B):
        stats = res_a[b]["stats"]                      # [32, 512]
        Pt, St = _extract_PS(stats[:, :256])
        Ps, Ss = _extract_PS(stats[:, 256:])
        R, T = _procrustes(Pt, St, Ps, Ss)
        rmat = np.zeros((128, 12), np.float32)
        for d in range(D):
            rmat[:, 4 * d : 4 * d + 3] = R[d].astype(np.float32)
            rmat[:, 4 * d + 3] = np.float32(T[d])
        in_maps_b.append({
            "src3": source_features[b].reshape(D, 128, 512),
            "rmat": rmat,
        })
    bkr_b = run_bass_kernel_spmd(ncb, in_maps_b, list(range(NCORES)))
    res_b = bkr_b.results
    _CACHE["last_results"] = (bkr_a, bkr_b)

    out = np.empty((B, N, D), np.float32)
    for b in range(B):
        a3 = res_b[b]["aligned3"].reshape(D, N)        # [d, n]
        out[b] = a3.T
    return out


# revision 20
# speedup vs baseline: 1.0089x; 1.0089x over previous
"""DeepGMR (segment_reduce) Trainium2 kernel.

Problem: B=8, D=3, N=65536, K=64.
    logits = W^T @ pts  (per batch, both template & source)
    gamma  = softmax over N
    mu_k   = sum_n gamma * pts / Npi ; 3x3 Procrustes; aligned = src @ R^T + T
sigma/Exx in the reference are dead code (never used by the output).

Sharding: batch-parallel, core c <-> batch b=c. No collectives.

Device kernel A (per core): sufficient statistics
    P[k, c] = sum_n exp(l[k, n]) * pts4[c, n],  pts4 = (x, y, z, 1)
  computed as:
    - logits via fp32r matmuls, stationary = W halves [3, 32] placed at
      row-band of the point quarter, 4 col-band-packed tiles per 512-pt
      step -> PSUM [128, 512] = (k-half x quarter-parity) x n
    - ACT exp over 3-bank chunks PSUM->SBUF bf16 (big free dim amortizes
      the +352cyc ACT overhead)
    - DVE 32x32 block transpose (bf16)
    - per 32-block matmuls: lhsT = host-prepared pts4 tiles [32, 16]
      (4-merged: 4 blocks -> out [16, 128], useful diagonal extracted on
      host), accumulated in one PSUM bank.
Host: mu = (P/S)/(1+2eps), exact reference Procrustes (SVD + det flip).
Device kernel B (per core): aligned = R @ src + T on DVE/ACT in
[128, 512] partition-major layout; host reassembles (B, N, 3).
"""

import numpy as np
import ml_dtypes

import concourse.bass as bass
import concourse.tile as tile
from concourse import bacc, mybir
from concourse.bass_utils import run_bass_kernel_spmd

B, D, N, K = 8, 3, 65536, 64
NCORES = 8
EPS = 1e-8
Q = N // 4            # 16384 points per quarter (per partition-group)
NSTEP = Q // 512      # 32 matmul steps per pass
CHUNK_STEPS = [2] * 16   # ACT/transpose chunk sizes (sum = 32)
assert sum(CHUNK_STEPS) == NSTEP

F32 = mybir.dt.float32
F32R = mybir.dt.float32r
BF16 = mybir.dt.bfloat16

_CACHE = {}


# --------------------------------------------------------------------------
# Kernel A: sufficient statistics
# --------------------------------------------------------------------------
def _build_kernel_a():
    nc = bacc.Bacc("TRN2", target_bir_lowering=False, debug=False,
                   num_devices=NCORES)
    tf_d = nc.dram_tensor("tf", [D, N], BF16, kind="ExternalInput").ap()
    sf_d = nc.dram_tensor("sf", [D, N], BF16, kind="ExternalInput").ap()
    w_d = nc.dram_tensor("w_rep", [8, 128], BF16, kind="ExternalInput").ap()
    p4_d = nc.dram_tensor("pts4m", [128, 16384], BF16, kind="ExternalInput").ap()
    stats_d = nc.dram_tensor("stats", [32, 512], F32, kind="ExternalOutput").ap()

    with tile.TileContext(nc) as tc:
        with (
            tc.tile_pool(name="const", bufs=1) as cpool,
            tc.tile_pool(name="stage", bufs=3) as spool,
            tc.tile_pool(name="esb", bufs=6) as epool,
            tc.tile_pool(name="btp", bufs=6) as bpool,
            tc.tile_pool(name="stat", bufs=1) as stpool,
            tc.tile_pool(name="lps", bufs=3, space=bass.MemorySpace.PSUM) as lpsum,
            tc.tile_pool(name="pacc", bufs=1, space=bass.MemorySpace.PSUM) as ppsum,
        ):
            w_sb = cpool.tile([8, 128], BF16)
            nc.sync.dma_start(out=w_sb[:, :], in_=w_d[:, :])
            p4z = cpool.tile([128, 128], BF16)
            nc.sync.dma_start(out=p4z[:, :], in_=p4_d[:, 0:128])
            zbias = cpool.tile([128, 1], F32)
            nc.vector.memset(zbias[:, :], 0.0)
            z64 = cpool.tile([128, 32], BF16)
            nc.vector.memset(z64[:, :], 0.0)
            stats_sb = stpool.tile([32, 512], F32)

            for t, feat_d in enumerate((tf_d, sf_d)):
                pacc0 = ppsum.tile([32, 128], F32, tag="pacc0")
                pacc1 = ppsum.tile([32, 128], F32, tag="pacc1")
                nc.tensor.matmul(pacc0[:, :], z64[0:64, :], p4z[0:64, 0:128],
                                 start=True, stop=False, skip_group_check=True)
                nc.tensor.matmul(pacc1[:, :], z64[64:128, :], p4z[64:128, 0:128],
                                 start=True, stop=False, skip_group_check=True,
                                 tile_position=(64, 0))
                for sc in range(2):        # superchunks of 16384 per half
                    # halves of N at partition rows 0-2 (H0) and 4-6 (H1)
                    stage = spool.tile([128, Q], BF16, tag="stage")
                    nc.sync.dma_start(
                        out=stage[0:3, :],
                        in_=feat_d[:, Q * sc : Q * (sc + 1)])
                    nc.sync.dma_start(
                        out=stage[4:7, :],
                        in_=feat_d[:, 2 * Q + Q * sc : 2 * Q + Q * (sc + 1)])
                    # row 3 is a gap in the [7, N] contract; W row 3 is zero
                    # but the data must be finite (0*Inf = NaN), so fill it.
                    nc.sync.dma_start(
                        out=stage[3:4, :], in_=feat_d[0:1, 0:Q])
                    p4p = spool.tile([128, 4096], BF16, tag="p4p")
                    nc.sync.dma_start(
                        out=p4p[:, :],
                        in_=p4_d[:, 4096 * (2 * t + sc) : 4096 * (2 * t + sc + 1)])
                    s0 = 0
                    for csteps in CHUNK_STEPS:
                        nfree = 512 * csteps
                        chunk = lpsum.tile([128, 1024], F32, tag="lchunk")
                        # logits: ONE [7,128]x[7,512] matmul per 512-pt step
                        # (block-diagonal W: rows 0-2 -> k cols 0-63 for H0,
                        #  rows 4-6 -> cols 64-127 for H1)
                        for sl in range(csteps):
                            s = s0 + sl
                            fs = slice(512 * s, 512 * (s + 1))
                            ps = slice(512 * sl, 512 * (sl + 1))
                            nc.tensor.matmul(
                                chunk[:, ps],
                                w_sb[0:7, :], stage[0:7, fs],
                                start=True, stop=True)
                        e_sb = epool.tile([128, 1024], BF16, tag="esb")
                        nc.scalar.activation(
                            e_sb[:, :nfree], chunk[:, :nfree],
                            mybir.ActivationFunctionType.Exp,
                            bias=zbias[:, :])
                        bt = bpool.tile([128, 1024], BF16, tag="btp")
                        nc.vector.transpose(bt[:, :nfree], e_sb[:, :nfree])
                        # P: contract 64 (both k-halves), block-diag pts4
                        # tiles [64, 32], rhs [64, 128], out [32, 128]
                        for b4 in range(4 * csteps):
                            t4 = 4 * s0 + b4
                            col = 32 * t4
                            for h, pac in ((0, pacc0), (1, pacc1)):
                                nc.tensor.matmul(
                                    pac[0:32, 0:128],
                                    p4p[64 * h : 64 * h + 64, col : col + 32],
                                    bt[64 * h : 64 * h + 64,
                                       128 * b4 : 128 * (b4 + 1)],
                                    start=False, stop=False,
                                    skip_group_check=True,
                                    tile_position=(64 * h, 0))
                        s0 += csteps
                nc.vector.tensor_copy(stats_sb[:, 256 * t : 256 * t + 128],
                                      pacc0[:, :])
                nc.vector.tensor_copy(stats_sb[:, 256 * t + 128 : 256 * t + 256],
                                      pacc1[:, :])
            nc.sync.dma_start(out=stats_d[:, :], in_=stats_sb[:, :])
    nc.compile()
    return nc


# --------------------------------------------------------------------------
# Kernel B: aligned = R @ src + T
# --------------------------------------------------------------------------
def _build_kernel_b():
    nc = bacc.Bacc("TRN2", target_bir_lowering=False, debug=False,
                   num_devices=NCORES)
    src_d = nc.dram_tensor("src3", [D, 128, 512], F32, kind="ExternalInput").ap()
    r_d = nc.dram_tensor("rmat", [128, 12], F32, kind="ExternalInput").ap()
    out_d = nc.dram_tensor("aligned3", [D, 128, 512], F32,
                           kind="ExternalOutput").ap()

    with tile.TileContext(nc) as tc:
        with tc.tile_pool(name="sb", bufs=1) as pool:
            r_sb = pool.tile([128, 12], F32)
            nc.sync.dma_start(out=r_sb[:, :], in_=r_d[:, :])
            comp = []
            for c in range(D):
                ct = pool.tile([128, 512], F32, tag=f"in{c}")
                nc.sync.dma_start(out=ct[:, :], in_=src_d[c, :, :])
                comp.append(ct)
            for d in range(D):
                acc = pool.tile([128, 512], F32, tag=f"acc{d}")
                # acc = (x * R[d,0]) + T[d]  (one DVE tensor_scalar, 2 scalars)
                nc.vector.tensor_scalar(
                    acc[:, :], comp[0][:, :],
                    r_sb[:, 4 * d : 4 * d + 1],
                    r_sb[:, 4 * d + 3 : 4 * d + 4],
                    op0=mybir.AluOpType.mult,
                    op1=mybir.AluOpType.add)
                for e in (1, 2):
                    # acc = (comp[e] * R[d,e]) + acc
                    nc.vector.scalar_tensor_tensor(
                        acc[:, :], comp[e][:, :],
                        r_sb[:, 4 * d + e : 4 * d + e + 1],
                        acc[:, :],
                        op0=mybir.AluOpType.mult,
                        op1=mybir.AluOpType.add)
                nc.sync.dma_start(out=out_d[d, :, :], in_=acc[:, :])
    nc.compile()
    return nc


# --------------------------------------------------------------------------
# Host-side helpers
# --------------------------------------------------------------------------
def _make_pts4m(feats):
    """feats (D, N) fp32 -> [128, 8192] bf16: block-diag [64, 32] pts4
    tiles (rows 0-31 -> cols 0-15 for kh0, rows 32-63 -> cols 16-31)."""
    pts4 = np.concatenate([feats, np.ones((1, N), np.float32)], axis=0)  # (4,N)
    out = np.zeros((128, 8192), dtype=np.float32)
    for h in range(2):
        for sc in range(2):
            q = 2 * h + sc
            qd = pts4[:, Q * q : Q * (q + 1)]          # (4, 16384)
            X = qd.reshape(4, 128, 4, 32)              # [c, T4, m, j]
            vals = X.transpose(3, 1, 2, 0).reshape(32, 128, 16)  # [j, T4, mc]
            blk = np.zeros((64, 128, 32), np.float32)
            blk[0:32, :, 0:16] = vals
            blk[32:64, :, 16:32] = vals
            out[64 * h : 64 * h + 64, 4096 * sc : 4096 * (sc + 1)] = (
                blk.reshape(64, 4096))
    return out.astype(ml_dtypes.bfloat16)


def _extract_PS(stats_half):
    """stats_half [32, 256] fp32 (bank H0 | bank H1) -> P (K, 3), S (K,)."""
    P4 = np.zeros((K, 4), np.float64)
    st = stats_half.astype(np.float64)
    for h in range(2):
        reg = st[:, 128 * h : 128 * h + 128]           # [32, 128]
        for khp in range(2):
            for m in range(4):
                blk = reg[16 * khp + 4 * m : 16 * khp + 4 * (m + 1),
                          32 * m : 32 * (m + 1)]       # [4c, 32i]
                P4[32 * khp : 32 * (khp + 1), :] += blk.T
    return P4[:, :3], P4[:, 3]


def _procrustes(Pt, St, Ps, Ss):
    """Exact reference tail. Returns R (3,3), T (3,)."""
    def mu_pi(P, S):
        mu = (P / S[:, None]) / (1.0 + 2 * EPS)
        pi = np.full(K, (1.0 + EPS) / N)
        return mu, pi
    t_mu, t_pi = mu_pi(Pt, St)
    s_mu, s_pi = mu_pi(Ps, Ss)
    cx = (s_mu * s_pi[:, None]).sum(0)
    cy = (t_mu * t_pi[:, None]).sum(0)
    mx = s_mu - cx
    my = t_mu - cy
    Wm = np.einsum('kd,ke->de', s_pi[:, None] * my, mx)
    U, _, Vh = np.linalg.svd(Wm)
    R0 = U @ Vh
    det = np.linalg.det(R0)
    Vh2 = Vh.copy()
    Vh2[:, -1] *= np.sign(det)
    R = U @ Vh2
    T = cy - R @ cx
    return R, T


def kernel(template_features, source_features, W):
    template_features = np.ascontiguousarray(
        np.asarray(template_features, dtype=np.float32))
    source_features = np.ascontiguousarray(
        np.asarray(source_features, dtype=np.float32))
    W = np.asarray(W, dtype=np.float32)

    if "a" not in _CACHE:
        _CACHE["a"] = _build_kernel_a()
        _CACHE["b"] = _build_kernel_b()
    nca, ncb = _CACHE["a"], _CACHE["b"]

    w_rep = np.zeros((8, 128), np.float32)
    w_rep[0:3, 0:64] = W
    w_rep[4:7, 64:128] = W
    w_rep = w_rep.astype(ml_dtypes.bfloat16)
    in_maps_a = []
    for b in range(B):
        in_maps_a.append({
            "tf": template_features[b].astype(ml_dtypes.bfloat16),
            "sf": source_features[b].astype(ml_dtypes.bfloat16),
            "w_rep": w_rep,
            "pts4m": np.concatenate(
                [_make_pts4m(template_features[b]),
                 _make_pts4m(source_features[b])], axis=1),
        })
    bkr_a = run_bass_kernel_spmd(nca, in_maps_a, list(range(NCORES)))
    res_a = bkr_a.results

    in_maps_b = []
    for b in range(B):
        stats = res_a[b]["stats"]                      # [32, 512]
        Pt, St = _extract_PS(stats[:, :256])
        Ps, Ss = _extract_PS(stats[:, 256:])
        R, T = _procrustes(Pt, St, Ps, Ss)
        rmat = np.zeros((128, 12), np.float32)
        for d in range(D):
            rmat[:, 4 * d : 4 * d + 3] = R[d].astype(np.float32)
            rmat[:, 4 * d + 3] = np.float32(T[d])
        in_maps_b.append({
            "src3": source_features[b].reshape(D, 128, 512),
            "rmat": rmat,
        })
    bkr_b = run_bass_kernel_spmd(ncb, in_maps_b, list(range(NCORES)))
    res_b = bkr_b.results
    _CACHE["last_results"] = (bkr_a, bkr_b)

    out = np.empty((B, N, D), np.float32)
    for b in range(B):
        a3 = res_b[b]["aligned3"].reshape(D, N)        # [d, n]
        out[b] = a3.T
    return out
